# revision 46
# baseline (speedup 1.0000x reference)
"""Causal attention layer on 8 TRN2 NeuronCores, data-parallel over batch.

Per-core problem (batch element n = core id):
    q = query @ Wq.T ; k = key @ Wk.T              (folded: G = Wk^T Wq)
    scores[s,t] = q[s]·k[t]  for t <= s            (f32r)
    attn = softmax(32 * scores)  (the +1 additive mask cancels in softmax;
                                  -inf masking == skipping t > s)
    out[s,:] = (attn @ vp) / rowsum,  vp = value @ Wv.T  (fp16, pre-projected)

Structure: one continuous PE burst.
  Prologue: G = Wk^T@Wq (direct-DMA'd weights), M = G@key^T (the "kt"
  tensor: scores = query @ M), vp = value@Wv^T in fp16 (value and Wv are
  transposed by the DMA xbar, not the PE).
  Strips (descending 15..0 so the wind-down strip is tiny): pass A
  computes scores chunks + row max (PE+DVE); pass B exponentiates (ACT),
  transposes attn via DMA xbar, and accumulates attn@vp (PE), then
  normalizes by the exp row-sum on the way out (ACT) and stores.
All f32 PE transposes are done in f32r (1.5 vs 2.0 cycles/row).
"""
import numpy as np
from contextlib import ExitStack

import concourse.bass as bass
import concourse.tile as tile
from concourse import bacc, mybir
from concourse.bass_utils import run_bass_kernel_spmd
from concourse.masks import make_identity

F32 = mybir.dt.float32
F32R = mybir.dt.float32r
FP16 = mybir.dt.float16

N, S, T, D = 8, 2048, 2048, 1024
P = 128
NSTRIP = S // P          # 16 query strips
TCH = 512                # t-chunk for score matmuls
OC = D // P              # 8 chunks of the projection/feature dim
SCALE = float(np.sqrt(np.float32(D)))  # 32.0
NEG = -1.0e30

PHASE_MARKS = []


def _mark(nc, label):
    PHASE_MARKS.append((label, nc.next_id()))


def _mm(nc, out, lhsT, rhs, dt, **kw):
    nc.tensor.matmul(out, lhsT.bitcast(dt), rhs.bitcast(dt), **kw)


def build_nc():
    PHASE_MARKS.clear()
    nc = bacc.Bacc("TRN2", target_bir_lowering=False, debug=False,
                   enable_asserts=False)
    q_d = nc.dram_tensor("query", [S, D], F32, kind="ExternalInput")
    k_d = nc.dram_tensor("key", [T, D], F32, kind="ExternalInput")
    v_d = nc.dram_tensor("value", [T, D], F32, kind="ExternalInput")
    wq_d = nc.dram_tensor("Wq", [D, D], F32, kind="ExternalInput")
    wk_d = nc.dram_tensor("Wk", [D, D], F32, kind="ExternalInput")
    wv_d = nc.dram_tensor("Wv", [D, D], F32, kind="ExternalInput")
    out_d = nc.dram_tensor("out", [S, D], F32, kind="ExternalOutput")

    with tile.TileContext(nc) as tc, ExitStack() as ctx:
        # ---- persistent pools ----
        const = ctx.enter_context(tc.tile_pool(name="const", bufs=1))
        kt_pool = ctx.enter_context(tc.tile_pool(name="kt", bufs=1))
        val_pool = ctx.enter_context(tc.tile_pool(name="val", bufs=1))
        st_pool = ctx.enter_context(tc.tile_pool(name="stats", bufs=24))
        mm_ps = ctx.enter_context(tc.tile_pool(name="mmps", bufs=4, space="PSUM"))
        ctx_ps = ctx.enter_context(tc.tile_pool(name="ctxps", bufs=2, space="PSUM"))

        ident = const.tile([P, P], F32)
        make_identity(nc, ident)
        ident16 = const.tile([P, P], FP16)
        nc.vector.tensor_copy(ident16[:], ident[:])

        def _tr(out, in_):
            nc.tensor.transpose(out, in_, ident[:])

        # the projected value vp = value @ Wv^T, fp16 [t_loc, tb, i]
        val = val_pool.tile([P, T // P, D], FP16)

        with ExitStack() as pro:
            wt_pool = pro.enter_context(tc.tile_pool(name="wt", bufs=1))
            wvt_pool = pro.enter_context(tc.tile_pool(name="wvt", bufs=1))
            vt_pool = pro.enter_context(tc.tile_pool(name="vt", bufs=3))
            hstage = pro.enter_context(tc.tile_pool(name="hstage", bufs=2))
            vstage = pro.enter_context(tc.tile_pool(name="vstage", bufs=2))
            bstage = pro.enter_context(tc.tile_pool(name="bstage", bufs=2))
            int_pool = pro.enter_context(tc.tile_pool(name="inT", bufs=1))
            gstage = pro.enter_context(tc.tile_pool(name="gstage", bufs=2))
            kstage = pro.enter_context(tc.tile_pool(name="kstage", bufs=3))

            # ---- loads + casts for the whole prologue, emitted up front so
            # every queue streams independently of PE progress ----
            _mark(nc, 'phaseG')
            # weights for G: DMA -> DVE cast (f32r requires a rounding
            # producer).  wq on the scalar queue, wk on gpsimd.
            # wq chain: DMA (scalar) -> cast (DVE); wk chain: DMA (gpsimd)
            # -> cast (ACT).  Separate staging pools so the two chains never
            # gate each other through slot recycling.
            wqwk = kt_pool.tile([P, 16, D], F32R, name="wqwk", tag="kt")
            for r in range(OC):
                stq = gstage.tile([P, D], F32, name="gstage", tag="gstage")
                nc.scalar.dma_start(stq[:], wq_d.ap()[r * P:(r + 1) * P, :])
                nc.vector.tensor_copy(wqwk[:, r, :], stq[:])
                stk = kstage.tile([P, D], F32, name="kstage", tag="kstage")
                nc.gpsimd.dma_start(stk[:], wk_d.ap()[r * P:(r + 1) * P, :])
                nc.scalar.activation(wqwk[:, OC + r, :], stk[:],
                                     mybir.ActivationFunctionType.Copy)

            # Wv strips: DMA (scalar) + fp16 cast (DVE, 2x rate on 16-bit);
            # transposed by PE filler units during G.
            wvT = wvt_pool.tile([P, OC, D], FP16, name="wvT")
            wv_bf = []
            for r in range(OC):
                stg = vstage.tile([P, D], F32, name="vstage", tag="vstage")
                nc.scalar.dma_start(stg[:], wv_d.ap()[r * P:(r + 1) * P, :])
                wbf = bstage.tile([P, D], FP16, name="bstage", tag="bstage")
                nc.vector.tensor_copy(wbf[:], stg[:])
                wv_bf.append(wbf)

            # value blocks 0..3: DMA (gpsimd) + fp16 cast; the rest are
            # emitted inside the vp loop (their casts chase the PE there).
            val_bf = {}

            def emit_val_load(tb, dma_eng=None, cast_eng='act'):
                vstg = vstage.tile([P, D], F32, name="vstage", tag="vstage")
                (dma_eng or nc.gpsimd).dma_start(
                    vstg[:], v_d.ap()[tb * P:(tb + 1) * P, :])
                vbf = bstage.tile([P, D], FP16, name="bstage", tag="bstage")
                if cast_eng == 'act':
                    nc.scalar.activation(vbf[:], vstg[:],
                                         mybir.ActivationFunctionType.Copy)
                else:
                    nc.vector.tensor_copy(vbf[:], vstg[:])
                val_bf[tb] = vbf

            for tb in range(4):
                emit_val_load(tb)

            # ---- PE filler units: fp16 transposes of Wv / value blocks and
            # the f32 transposes of the first key sub-chunks, interleaved
            # between G's accumulation steps to absorb weight-cast latency ----
            KSUB = 256            # M works in 256-col sub-chunks (2 xt bufs
            val_t = {}            # ping-pong so transposes overlap matmuls)
            kxt = {}

            def emit_ktr_sub(n):
                """key[n*256:(n+1)*256, :] -> xt tile [128(i), OC, 256]."""
                xt = int_pool.tile([P, OC, KSUB], F32R, name="inT", tag="inT")
                for sl in range(2):
                    stg = hstage.tile([P, D], F32, name="hstage", tag="hstage")
                    nc.sync.dma_start(
                        stg[:],
                        k_d.ap()[n * KSUB + sl * P:n * KSUB + (sl + 1) * P, :])
                    for g in range(0, OC, 4):
                        ps = ctx_ps.tile([P, 2, TCH], F32, name="ctps",
                                         tag="ctxps")
                        for c in range(4):
                            _tr(ps[:, 0, c * P:(c + 1) * P],
                                stg[:, (g + c) * P:(g + c + 1) * P])
                        nc.vector.tensor_copy(
                            xt[:, g:g + 4, sl * P:(sl + 1) * P],
                            ps[:, 0, :].rearrange("p (c s) -> p c s", c=4))
                kxt[n] = xt

            def emit_wv_tr(r):
                wbf = wv_bf[r]
                ps = ctx_ps.tile([P, 2, TCH], FP16, name="ctps", tag="ctxps")
                for jc in range(OC):
                    nc.tensor.transpose(
                        ps[:, jc // 4, (jc % 4) * P:(jc % 4 + 1) * P],
                        wbf[:, jc * P:(jc + 1) * P], ident16[:])
                nc.vector.tensor_copy(
                    wvT[:, :, r * P:(r + 1) * P],
                    ps.rearrange("p a (c s) -> p (a c) s", c=4))

            def emit_val_tr(tb):
                vbf = val_bf.pop(tb)
                vt = vt_pool.tile([P, OC, P], FP16, name="vt", tag="vt")
                ps = ctx_ps.tile([P, 2, TCH], FP16, name="ctps", tag="ctxps")
                for jc in range(OC):
                    nc.tensor.transpose(
                        ps[:, jc // 4, (jc % 4) * P:(jc % 4 + 1) * P],
                        vbf[:, jc * P:(jc + 1) * P], ident16[:])
                nc.vector.tensor_copy(
                    vt[:, :, :], ps.rearrange("p a (c s) -> p (a c) s", c=4))
                val_t[tb] = vt

            filler = [lambda n=n: emit_ktr_sub(n) for n in range(2)]
            filler += [lambda r=r: emit_wv_tr(r) for r in range(OC)]
            filler += [lambda tb=tb: emit_val_tr(tb) for tb in range(3)]

            # ---- phase G matmuls: 4 quarter-passes of 4 psum accumulators
            # (leaves ctx_ps free for the filler transposes) ----
            gT = wt_pool.tile([P, OC, D], F32R, name="wT", tag="wT")
            for ih in range(2):
                for jcg in range(2):
                    accs = [mm_ps.tile([P, TCH], F32, name=f"ga{j}",
                                       tag="mmps")[:] for j in range(4)]
                    for oc in range(OC):
                        for j in range(4):
                            jc = jcg * 4 + j
                            _mm(nc, accs[j],
                                wqwk[:, OC + oc, jc * P:(jc + 1) * P],
                                wqwk[:, oc, ih * TCH:(ih + 1) * TCH], F32R,
                                start=(oc == 0), stop=(oc == OC - 1))
                        for _ in range(2):
                            if filler:
                                filler.pop(0)()
                    for j in range(4):
                        jc = jcg * 4 + j
                        nc.scalar.activation(
                            gT[:, jc, ih * TCH:(ih + 1) * TCH], accs[j],
                            mybir.ActivationFunctionType.Copy)
            while filler:
                filler.pop(0)()

            # ---- phase M: M = G @ key^T -> resident SBUF [128, OC, T] ----
            # (reuses the wqwk slot; M plays the role kT did: scores =
            # query @ M).  256-col sub-chunks with 2 xt buffers: the next
            # sub-chunk's transposes+DMA fully overlap this one's matmuls.
            _mark(nc, 'phaseM')
            kt = kt_pool.tile([P, OC, T], F32R, name="kt", tag="kt")
            for n in range(T // KSUB):
                if n + 1 < T // KSUB and n + 1 not in kxt:
                    emit_ktr_sub(n + 1)
                kT_in = kxt.pop(n)
                for ic in range(OC):
                    ps = mm_ps.tile([P, KSUB], F32, name="pjps", tag="mmps")
                    for jc in range(OC):
                        _mm(nc, ps[:], gT[:, jc, ic * P:(ic + 1) * P],
                            kT_in[:, jc, :], F32R,
                            start=(jc == 0), stop=(jc == OC - 1))
                    # ACT does the psum->sbuf copies here: the DVE is busy
                    # with the xt copies and fp16 casts during phase M
                    nc.scalar.activation(
                        kt[:, ic, n * KSUB:(n + 1) * KSUB], ps[:],
                        mybir.ActivationFunctionType.Copy)

            # ---- phase V: vp = value @ Wv^T in fp16 ----
            # val_t[tb] transposes (PE) run one block ahead of the vp
            # matmuls so the DVE copy latency is hidden.
            # value loads 4..9 staged on the sync queue (idle after the k
            # loads; gpsimd SWDGE issue is ~3x slower) with DVE casts --
            # ACT is still draining the kt copies when vp starts
            _mark(nc, 'phaseV')
            for tb in range(4, 10):
                emit_val_load(tb, dma_eng=nc.sync, cast_eng='dve')
            for tb in range(T // P):
                if 10 <= tb + 6 < T // P:
                    emit_val_load(tb + 6, dma_eng=nc.sync)
                vt = val_t.pop(tb)
                for ih in range(2):
                    ps = mm_ps.tile([P, TCH], F32, name="vpps", tag="mmps")
                    for jc in range(OC):
                        nc.tensor.matmul(
                            ps[:], vt[:, jc, :],
                            wvT[:, jc, ih * TCH:(ih + 1) * TCH],
                            start=(jc == 0), stop=(jc == OC - 1))
                    nc.scalar.activation(
                        val[:, tb, ih * TCH:(ih + 1) * TCH], ps[:],
                        mybir.ActivationFunctionType.Copy)
                # vt[tb] is consumed: its pool slot can take block tb+3
                if tb + 3 < T // P:
                    emit_val_tr(tb + 3)

        # ---- strip phase pools (prologue pools' SBUF is released) ----
        qstage = ctx.enter_context(tc.tile_pool(name="qstage", bufs=3))
        qts_pool = ctx.enter_context(tc.tile_pool(name="qts", bufs=3))
        sc_pool = ctx.enter_context(tc.tile_pool(name="scores", bufs=4))
        exp_pool = ctx.enter_context(tc.tile_pool(name="exp", bufs=3))
        at_pool = ctx.enter_context(tc.tile_pool(name="attnT", bufs=2))
        ob_pool = ctx.enter_context(tc.tile_pool(name="outb", bufs=2))
        cst2 = ctx.enter_context(tc.tile_pool(name="cst2", bufs=1))

        # Additive diag masks for the 4 possible strip positions inside a
        # 512-wide t-chunk: mask[j][x, y] = 0 if y <= j*128 + x else -1e30
        diagmask = cst2.tile([P, 4, TCH], F32)
        nc.gpsimd.memset(diagmask[:], 0.0)
        for j in range(4):
            nc.gpsimd.affine_select(
                out=diagmask[:, j, :], in_=diagmask[:, j, :],
                compare_op=mybir.AluOpType.is_ge, fill=NEG,
                base=j * P, channel_multiplier=1, pattern=[[-1, TCH]])

        q_tiles = {}

        def prefetch_q(si):
            stg = qstage.tile([P, D], F32, name="qstg", tag="qstg")
            nc.sync.dma_start(stg[:], q_d.ap()[si * P:(si + 1) * P, :])
            q_tiles[si] = stg

        state = {}

        def pass_a(si):
            """scores chunks + row max for strip si (PE + DVE)."""
            _mark(nc, f'strip{si}')
            stg = q_tiles.pop(si)
            qts = qts_pool.tile([P, OC, P], F32R, name="qts")
            for g in range(0, OC, 4):
                ps = mm_ps.tile([P, TCH], F32, name="tps", tag="mmps")
                for c in range(4):
                    _tr(ps[:, c * P:(c + 1) * P],
                        stg[:, (g + c) * P:(g + c + 1) * P])
                nc.vector.tensor_copy(
                    qts[:, g:g + 4, :],
                    ps.rearrange("p (c s) -> p c s", c=4))

            scores = sc_pool.tile([P, S], F32, name="scores")
            # diag chunk first so its (mask + max) tail overlaps the other
            # chunks' matmuls; per-chunk maxes keep the reduce off the
            # critical path.  f32r needs moving dim >= 256.
            nch = si // 4 + 1
            dw = max(2 * P, (si % 4 + 1) * P)
            cmaxes = []
            for c in [nch - 1] + list(range(nch - 1)):
                cw = dw if c == nch - 1 else TCH
                ps = mm_ps.tile([P, TCH], F32, name="scps", tag="mmps")
                for oc in range(OC):
                    _mm(nc, ps[:, :cw], qts[:, oc, :],
                        kt[:, oc, c * TCH:c * TCH + cw], F32R,
                        start=(oc == 0), stop=(oc == OC - 1))
                dst = scores[:, c * TCH:c * TCH + cw]
                if c < nch - 1:
                    nc.vector.tensor_copy(dst, ps[:])
                else:
                    nc.vector.tensor_add(dst, ps[:, :cw],
                                         diagmask[:, si % 4, :cw])
                cmax = st_pool.tile([P, 1], F32, name="cmax", tag="st")
                nc.vector.reduce_max(cmax[:], dst, axis=mybir.AxisListType.X)
                cmaxes.append(cmax)
            while len(cmaxes) > 1:
                nxt = []
                for a, b in zip(cmaxes[::2], cmaxes[1::2]):
                    m = st_pool.tile([P, 1], F32, name="cmax", tag="st")
                    nc.vector.tensor_max(m[:], a[:], b[:])
                    nxt.append(m)
                if len(cmaxes) % 2:
                    nxt.append(cmaxes[-1])
                cmaxes = nxt
            negm = st_pool.tile([P, 1], F32, name="negm", tag="st")
            nc.vector.tensor_scalar_mul(negm[:], cmaxes[0][:], -SCALE)
            state[si] = (scores, negm)

        def pass_b(si):
            """exp (ACT) + xbar transpose + attn@vp (PE) + store."""
            s0 = si * P
            nch = si // 4 + 1
            ntb = si + 1
            dw = max(2 * P, (si % 4 + 1) * P)
            scores, negm = state.pop(si)

            attnT = at_pool.tile([P, NSTRIP, P], FP16, name="attnT")
            cps = ctx_ps.tile([P, 2, TCH], F32, name="ctxps", tag="ctxps")
            # all exps first (they only need negm); then the transpose of
            # chunk c+1 overlaps the AV matmuls of chunk c
            partials = []
            expcs = []
            for c in range(nch):
                cw = dw if c == nch - 1 else TCH
                expc = exp_pool.tile([P, TCH], FP16, name="expc")
                # no accum_out: its ~280ns readback would delay the attnT
                # transposes that consume expc; the DVE sums instead
                nc.scalar.activation(expc[:, :cw],
                                     scores[:, c * TCH:c * TCH + cw],
                                     mybir.ActivationFunctionType.Exp,
                                     bias=negm[:], scale=SCALE)
                part = st_pool.tile([P, 1], F32, name="part", tag="st")
                nc.vector.reduce_sum(part[:], expc[:, :cw],
                                     axis=mybir.AxisListType.X)
                partials.append(part)
                expcs.append(expc)

            def tr_chunk(c):
                nblk = min(4, ntb - 4 * c)
                ps = mm_ps.tile([P, TCH], FP16, name="tps2", tag="mmps")
                for g in range(nblk):
                    nc.tensor.transpose(ps[:, g * P:(g + 1) * P],
                                        expcs[c][:, g * P:(g + 1) * P],
                                        ident16[:])
                # ACT (not DVE) copy: keeps the exp->transpose->AV chain off
                # the DVE queue, which is busy with the next strip's pass A
                nc.scalar.activation(
                    attnT[:, 4 * c:4 * c + nblk, :],
                    ps[:, :nblk * P].rearrange("p (c s) -> p c s", c=nblk),
                    mybir.ActivationFunctionType.Copy)

            tr_chunk(0)
            for c in range(nch):
                if c + 1 < nch:
                    tr_chunk(c + 1)
                nblk = min(4, ntb - 4 * c)
                for tb in range(4 * c, 4 * c + nblk):
                    for ih in range(2):
                        nc.tensor.matmul(cps[:, ih, :],
                                         attnT[:, tb, :],
                                         val[:, tb, ih * TCH:(ih + 1) * TCH],
                                         start=(tb == 0),
                                         stop=(tb == ntb - 1))
            rowsum = st_pool.tile([P, 1], F32, name="rowsum", tag="st")
            if len(partials) == 1:
                nc.vector.tensor_copy(rowsum[:], partials[0][:])
            else:
                nc.vector.tensor_add(rowsum[:], partials[0][:], partials[1][:])
                for part in partials[2:]:
                    nc.vector.tensor_add(rowsum[:], rowsum[:], part[:])
            recip = st_pool.tile([P, 1], F32, name="recip", tag="st")
            nc.vector.reciprocal(recip[:], rowsum[:])
            ob = ob_pool.tile([P, D], F32, name="ob")
            nc.scalar.activation(ob[:], cps.rearrange("p a b -> p (a b)"),
                                 mybir.ActivationFunctionType.Copy,
                                 scale=recip[:])
            nc.sync.dma_start(out_d.ap()[s0:s0 + P, :], ob[:])

        # ---- strip loop: descending order (big strips first, so the
        # wind-down tail is the cheapest strip); pipeline depth 2, deepening
        # to 3 for the small tail strips whose serial chains dominate ----
        # strip 2 (tiny) first: its short softmax chain fills the pipeline
        # right after the prologue; then big-to-small so the wind-down strip
        # is the cheapest
        order = [2] + [s for s in range(NSTRIP - 1, -1, -1) if s != 2]
        prefetch_q(order[0])
        prefetch_q(order[1])
        prefetch_q(order[2])
        pass_a(order[0])
        pass_a(order[1])
        pass_a(order[2])
        nxt = 3
        for i, si in enumerate(order):
            want = nxt + 1 if i >= 9 else nxt
            for k in range(nxt, min(want + 1, NSTRIP)):
                prefetch_q(order[k])
            pass_b(si)
            for k in range(nxt, min(want + 1, NSTRIP)):
                pass_a(order[k])
                nxt = k + 1

    _mark(nc, 'end')
    nc.finalize()
    return nc


_NC_CACHE = None


def kernel(**inputs):
    global _NC_CACHE
    if _NC_CACHE is None:
        _NC_CACHE = build_nc()
    nc = _NC_CACHE
    query = np.ascontiguousarray(inputs["query"], dtype=np.float32)
    key = np.ascontiguousarray(inputs["key"], dtype=np.float32)
    value = np.ascontiguousarray(inputs["value"], dtype=np.float32)
    Wq = np.ascontiguousarray(inputs["Wq"], dtype=np.float32)
    Wk = np.ascontiguousarray(inputs["Wk"], dtype=np.float32)
    Wv = np.ascontiguousarray(inputs["Wv"], dtype=np.float32)
    in_maps = [
        {"query": query[i], "key": key[i], "value": value[i],
         "Wq": Wq, "Wk": Wk, "Wv": Wv}
        for i in range(N)
    ]
    res = run_bass_kernel_spmd(nc, in_maps, core_ids=list(range(N)))
    return np.stack([res.results[i]["out"] for i in range(N)], axis=0)


# revision 48
# speedup vs baseline: 1.0051x; 1.0051x over previous
"""Causal attention layer on 8 TRN2 NeuronCores, data-parallel over batch.

Per-core problem (batch element n = core id):
    q = query @ Wq.T ; k = key @ Wk.T              (folded: G = Wk^T Wq)
    scores[s,t] = q[s]·k[t]  for t <= s            (f32r)
    attn = softmax(32 * scores)  (the +1 additive mask cancels in softmax;
                                  -inf masking == skipping t > s)
    out[s,:] = (attn @ vp) / rowsum,  vp = value @ Wv.T  (fp16, pre-projected)

Structure: one continuous PE burst.
  Prologue: G = Wk^T@Wq (direct-DMA'd weights), M = G@key^T (the "kt"
  tensor: scores = query @ M), vp = value@Wv^T in fp16 (value and Wv are
  transposed by the DMA xbar, not the PE).
  Strips (descending 15..0 so the wind-down strip is tiny): pass A
  computes scores chunks + row max (PE+DVE); pass B exponentiates (ACT),
  transposes attn via DMA xbar, and accumulates attn@vp (PE), then
  normalizes by the exp row-sum on the way out (ACT) and stores.
All f32 PE transposes are done in f32r (1.5 vs 2.0 cycles/row).
"""
import numpy as np
from contextlib import ExitStack

import concourse.bass as bass
import concourse.tile as tile
from concourse import bacc, mybir
from concourse.bass_utils import run_bass_kernel_spmd
from concourse.masks import make_identity

F32 = mybir.dt.float32
F32R = mybir.dt.float32r
FP16 = mybir.dt.float16

N, S, T, D = 8, 2048, 2048, 1024
P = 128
NSTRIP = S // P          # 16 query strips
TCH = 512                # t-chunk for score matmuls
OC = D // P              # 8 chunks of the projection/feature dim
SCALE = float(np.sqrt(np.float32(D)))  # 32.0
NEG = -1.0e30

PHASE_MARKS = []


def _mark(nc, label):
    PHASE_MARKS.append((label, nc.next_id()))


def _mm(nc, out, lhsT, rhs, dt, **kw):
    nc.tensor.matmul(out, lhsT.bitcast(dt), rhs.bitcast(dt), **kw)


def build_nc():
    PHASE_MARKS.clear()
    nc = bacc.Bacc("TRN2", target_bir_lowering=False, debug=False,
                   enable_asserts=False)
    q_d = nc.dram_tensor("query", [S, D], F32, kind="ExternalInput")
    k_d = nc.dram_tensor("key", [T, D], F32, kind="ExternalInput")
    v_d = nc.dram_tensor("value", [T, D], F32, kind="ExternalInput")
    wq_d = nc.dram_tensor("Wq", [D, D], F32, kind="ExternalInput")
    wk_d = nc.dram_tensor("Wk", [D, D], F32, kind="ExternalInput")
    wv_d = nc.dram_tensor("Wv", [D, D], F32, kind="ExternalInput")
    out_d = nc.dram_tensor("out", [S, D], F32, kind="ExternalOutput")

    with tile.TileContext(nc) as tc, ExitStack() as ctx:
        # ---- persistent pools ----
        const = ctx.enter_context(tc.tile_pool(name="const", bufs=1))
        kt_pool = ctx.enter_context(tc.tile_pool(name="kt", bufs=1))
        val_pool = ctx.enter_context(tc.tile_pool(name="val", bufs=1))
        st_pool = ctx.enter_context(tc.tile_pool(name="stats", bufs=24))
        mm_ps = ctx.enter_context(tc.tile_pool(name="mmps", bufs=4, space="PSUM"))
        ctx_ps = ctx.enter_context(tc.tile_pool(name="ctxps", bufs=2, space="PSUM"))

        ident = const.tile([P, P], F32)
        make_identity(nc, ident)
        ident16 = const.tile([P, P], FP16)
        nc.vector.tensor_copy(ident16[:], ident[:])

        def _tr(out, in_):
            nc.tensor.transpose(out, in_, ident[:])

        # the projected value vp = value @ Wv^T, fp16 [t_loc, tb, i]
        val = val_pool.tile([P, T // P, D], FP16)

        with ExitStack() as pro:
            wt_pool = pro.enter_context(tc.tile_pool(name="wt", bufs=1))
            wvt_pool = pro.enter_context(tc.tile_pool(name="wvt", bufs=1))
            vt_pool = pro.enter_context(tc.tile_pool(name="vt", bufs=3))
            hstage = pro.enter_context(tc.tile_pool(name="hstage", bufs=2))
            vstage = pro.enter_context(tc.tile_pool(name="vstage", bufs=2))
            bstage = pro.enter_context(tc.tile_pool(name="bstage", bufs=2))
            int_pool = pro.enter_context(tc.tile_pool(name="inT", bufs=1))
            gstage = pro.enter_context(tc.tile_pool(name="gstage", bufs=2))
            kstage = pro.enter_context(tc.tile_pool(name="kstage", bufs=3))

            # ---- loads + casts for the whole prologue, emitted up front so
            # every queue streams independently of PE progress ----
            _mark(nc, 'phaseG')
            # weights for G: DMA -> DVE cast (f32r requires a rounding
            # producer).  wq on the scalar queue, wk on gpsimd.
            # wq chain: DMA (scalar) -> cast (DVE); wk chain: DMA (gpsimd)
            # -> cast (ACT).  Separate staging pools so the two chains never
            # gate each other through slot recycling.
            wqwk = kt_pool.tile([P, 16, D], F32R, name="wqwk", tag="kt")
            for r in range(OC):
                stq = gstage.tile([P, D], F32, name="gstage", tag="gstage")
                nc.scalar.dma_start(stq[:], wq_d.ap()[r * P:(r + 1) * P, :])
                nc.vector.tensor_copy(wqwk[:, r, :], stq[:])
                stk = kstage.tile([P, D], F32, name="kstage", tag="kstage")
                nc.gpsimd.dma_start(stk[:], wk_d.ap()[r * P:(r + 1) * P, :])
                nc.scalar.activation(wqwk[:, OC + r, :], stk[:],
                                     mybir.ActivationFunctionType.Copy)

            # Wv strips: DMA (scalar) + fp16 cast (DVE, 2x rate on 16-bit);
            # transposed by PE filler units during G.
            wvT = wvt_pool.tile([P, OC, D], FP16, name="wvT")
            wv_bf = []
            for r in range(OC):
                stg = vstage.tile([P, D], F32, name="vstage", tag="vstage")
                nc.scalar.dma_start(stg[:], wv_d.ap()[r * P:(r + 1) * P, :])
                wbf = bstage.tile([P, D], FP16, name="bstage", tag="bstage")
                nc.vector.tensor_copy(wbf[:], stg[:])
                wv_bf.append(wbf)

            # value blocks 0..3: DMA (gpsimd) + fp16 cast; the rest are
            # emitted inside the vp loop (their casts chase the PE there).
            val_bf = {}

            def emit_val_load(tb, dma_eng=None, cast_eng='act'):
                vstg = vstage.tile([P, D], F32, name="vstage", tag="vstage")
                (dma_eng or nc.gpsimd).dma_start(
                    vstg[:], v_d.ap()[tb * P:(tb + 1) * P, :])
                vbf = bstage.tile([P, D], FP16, name="bstage", tag="bstage")
                if cast_eng == 'act':
                    nc.scalar.activation(vbf[:], vstg[:],
                                         mybir.ActivationFunctionType.Copy)
                else:
                    nc.vector.tensor_copy(vbf[:], vstg[:])
                val_bf[tb] = vbf

            for tb in range(4):
                emit_val_load(tb)

            # ---- PE filler units: fp16 transposes of Wv / value blocks and
            # the f32 transposes of the first key sub-chunks, interleaved
            # between G's accumulation steps to absorb weight-cast latency ----
            KSUB = 256            # M works in 256-col sub-chunks (2 xt bufs
            val_t = {}            # ping-pong so transposes overlap matmuls)
            kxt = {}

            def emit_ktr_sub(n):
                """key[n*256:(n+1)*256, :] -> xt tile [128(i), OC, 256]."""
                xt = int_pool.tile([P, OC, KSUB], F32R, name="inT", tag="inT")
                for sl in range(2):
                    stg = hstage.tile([P, D], F32, name="hstage", tag="hstage")
                    nc.sync.dma_start(
                        stg[:],
                        k_d.ap()[n * KSUB + sl * P:n * KSUB + (sl + 1) * P, :])
                    for g in range(0, OC, 4):
                        ps = ctx_ps.tile([P, 2, TCH], F32, name="ctps",
                                         tag="ctxps")
                        for c in range(4):
                            _tr(ps[:, 0, c * P:(c + 1) * P],
                                stg[:, (g + c) * P:(g + c + 1) * P])
                        nc.vector.tensor_copy(
                            xt[:, g:g + 4, sl * P:(sl + 1) * P],
                            ps[:, 0, :].rearrange("p (c s) -> p c s", c=4))
                kxt[n] = xt

            def emit_wv_tr(r):
                wbf = wv_bf[r]
                ps = ctx_ps.tile([P, 2, TCH], FP16, name="ctps", tag="ctxps")
                for jc in range(OC):
                    nc.tensor.transpose(
                        ps[:, jc // 4, (jc % 4) * P:(jc % 4 + 1) * P],
                        wbf[:, jc * P:(jc + 1) * P], ident16[:])
                nc.vector.tensor_copy(
                    wvT[:, :, r * P:(r + 1) * P],
                    ps.rearrange("p a (c s) -> p (a c) s", c=4))

            def emit_val_tr(tb):
                vbf = val_bf.pop(tb)
                vt = vt_pool.tile([P, OC, P], FP16, name="vt", tag="vt")
                ps = ctx_ps.tile([P, 2, TCH], FP16, name="ctps", tag="ctxps")
                for jc in range(OC):
                    nc.tensor.transpose(
                        ps[:, jc // 4, (jc % 4) * P:(jc % 4 + 1) * P],
                        vbf[:, jc * P:(jc + 1) * P], ident16[:])
                nc.vector.tensor_copy(
                    vt[:, :, :], ps.rearrange("p a (c s) -> p (a c) s", c=4))
                val_t[tb] = vt

            filler = [lambda n=n: emit_ktr_sub(n) for n in range(2)]
            filler += [lambda r=r: emit_wv_tr(r) for r in range(OC)]
            filler += [lambda tb=tb: emit_val_tr(tb) for tb in range(3)]

            # ---- phase G matmuls: 4 quarter-passes of 4 psum accumulators
            # (leaves ctx_ps free for the filler transposes) ----
            gT = wt_pool.tile([P, OC, D], F32R, name="wT", tag="wT")
            for ih in range(2):
                for jcg in range(2):
                    accs = [mm_ps.tile([P, TCH], F32, name=f"ga{j}",
                                       tag="mmps")[:] for j in range(4)]
                    for oc in range(OC):
                        for j in range(4):
                            jc = jcg * 4 + j
                            _mm(nc, accs[j],
                                wqwk[:, OC + oc, jc * P:(jc + 1) * P],
                                wqwk[:, oc, ih * TCH:(ih + 1) * TCH], F32R,
                                start=(oc == 0), stop=(oc == OC - 1))
                        for _ in range(2):
                            if filler:
                                filler.pop(0)()
                    for j in range(4):
                        jc = jcg * 4 + j
                        nc.scalar.activation(
                            gT[:, jc, ih * TCH:(ih + 1) * TCH], accs[j],
                            mybir.ActivationFunctionType.Copy)
            while filler:
                filler.pop(0)()

            # ---- phase M: M = G @ key^T -> resident SBUF [128, OC, T] ----
            # (reuses the wqwk slot; M plays the role kT did: scores =
            # query @ M).  256-col sub-chunks with 2 xt buffers: the next
            # sub-chunk's transposes+DMA fully overlap this one's matmuls.
            _mark(nc, 'phaseM')
            kt = kt_pool.tile([P, OC, T], F32R, name="kt", tag="kt")
            for n in range(T // KSUB):
                if n + 1 < T // KSUB and n + 1 not in kxt:
                    emit_ktr_sub(n + 1)
                kT_in = kxt.pop(n)
                for ic in range(OC):
                    ps = mm_ps.tile([P, KSUB], F32, name="pjps", tag="mmps")
                    for jc in range(OC):
                        _mm(nc, ps[:], gT[:, jc, ic * P:(ic + 1) * P],
                            kT_in[:, jc, :], F32R,
                            start=(jc == 0), stop=(jc == OC - 1))
                    # ACT does the psum->sbuf copies here: the DVE is busy
                    # with the xt copies and fp16 casts during phase M
                    nc.scalar.activation(
                        kt[:, ic, n * KSUB:(n + 1) * KSUB], ps[:],
                        mybir.ActivationFunctionType.Copy)

            # ---- phase V: vp = value @ Wv^T in fp16 ----
            # val_t[tb] transposes (PE) run one block ahead of the vp
            # matmuls so the DVE copy latency is hidden.
            # value loads 4..9 staged on the sync queue (idle after the k
            # loads; gpsimd SWDGE issue is ~3x slower) with DVE casts --
            # ACT is still draining the kt copies when vp starts
            _mark(nc, 'phaseV')
            for tb in range(4, 10):
                emit_val_load(tb, dma_eng=nc.sync, cast_eng='dve')
            for tb in range(T // P):
                if 10 <= tb + 6 < T // P:
                    emit_val_load(tb + 6, dma_eng=nc.sync)
                vt = val_t.pop(tb)
                for ih in range(2):
                    ps = mm_ps.tile([P, TCH], F32, name="vpps", tag="mmps")
                    for jc in range(OC):
                        nc.tensor.matmul(
                            ps[:], vt[:, jc, :],
                            wvT[:, jc, ih * TCH:(ih + 1) * TCH],
                            start=(jc == 0), stop=(jc == OC - 1))
                    nc.scalar.activation(
                        val[:, tb, ih * TCH:(ih + 1) * TCH], ps[:],
                        mybir.ActivationFunctionType.Copy)
                # vt[tb] is consumed: its pool slot can take block tb+3
                if tb + 3 < T // P:
                    emit_val_tr(tb + 3)

        # ---- strip phase pools (prologue pools' SBUF is released) ----
        qstage = ctx.enter_context(tc.tile_pool(name="qstage", bufs=3))
        qts_pool = ctx.enter_context(tc.tile_pool(name="qts", bufs=3))
        sc_pool = ctx.enter_context(tc.tile_pool(name="scores", bufs=5))
        exp_pool = ctx.enter_context(tc.tile_pool(name="exp", bufs=4))
        at_pool = ctx.enter_context(tc.tile_pool(name="attnT", bufs=3))
        ob_pool = ctx.enter_context(tc.tile_pool(name="outb", bufs=2))
        cst2 = ctx.enter_context(tc.tile_pool(name="cst2", bufs=1))

        # Additive diag masks for the 4 possible strip positions inside a
        # 512-wide t-chunk: mask[j][x, y] = 0 if y <= j*128 + x else -1e30
        diagmask = cst2.tile([P, 4, TCH], F32)
        nc.gpsimd.memset(diagmask[:], 0.0)
        for j in range(4):
            nc.gpsimd.affine_select(
                out=diagmask[:, j, :], in_=diagmask[:, j, :],
                compare_op=mybir.AluOpType.is_ge, fill=NEG,
                base=j * P, channel_multiplier=1, pattern=[[-1, TCH]])

        q_tiles = {}

        def prefetch_q(si):
            stg = qstage.tile([P, D], F32, name="qstg", tag="qstg")
            nc.sync.dma_start(stg[:], q_d.ap()[si * P:(si + 1) * P, :])
            q_tiles[si] = stg

        state = {}

        def pass_a(si):
            """scores chunks + row max for strip si (PE + DVE)."""
            _mark(nc, f'strip{si}')
            stg = q_tiles.pop(si)
            qts = qts_pool.tile([P, OC, P], F32R, name="qts")
            for g in range(0, OC, 4):
                ps = mm_ps.tile([P, TCH], F32, name="tps", tag="mmps")
                for c in range(4):
                    _tr(ps[:, c * P:(c + 1) * P],
                        stg[:, (g + c) * P:(g + c + 1) * P])
                nc.vector.tensor_copy(
                    qts[:, g:g + 4, :],
                    ps.rearrange("p (c s) -> p c s", c=4))

            scores = sc_pool.tile([P, S], F32, name="scores")
            # diag chunk first so its (mask + max) tail overlaps the other
            # chunks' matmuls; per-chunk maxes keep the reduce off the
            # critical path.  f32r needs moving dim >= 256.
            nch = si // 4 + 1
            dw = max(2 * P, (si % 4 + 1) * P)
            cmaxes = []
            for c in [nch - 1] + list(range(nch - 1)):
                cw = dw if c == nch - 1 else TCH
                ps = mm_ps.tile([P, TCH], F32, name="scps", tag="mmps")
                for oc in range(OC):
                    _mm(nc, ps[:, :cw], qts[:, oc, :],
                        kt[:, oc, c * TCH:c * TCH + cw], F32R,
                        start=(oc == 0), stop=(oc == OC - 1))
                dst = scores[:, c * TCH:c * TCH + cw]
                if c < nch - 1:
                    nc.vector.tensor_copy(dst, ps[:])
                else:
                    nc.vector.tensor_add(dst, ps[:, :cw],
                                         diagmask[:, si % 4, :cw])
                cmax = st_pool.tile([P, 1], F32, name="cmax", tag="st")
                nc.vector.reduce_max(cmax[:], dst, axis=mybir.AxisListType.X)
                cmaxes.append(cmax)
            while len(cmaxes) > 1:
                nxt = []
                for a, b in zip(cmaxes[::2], cmaxes[1::2]):
                    m = st_pool.tile([P, 1], F32, name="cmax", tag="st")
                    nc.vector.tensor_max(m[:], a[:], b[:])
                    nxt.append(m)
                if len(cmaxes) % 2:
                    nxt.append(cmaxes[-1])
                cmaxes = nxt
            negm = st_pool.tile([P, 1], F32, name="negm", tag="st")
            nc.vector.tensor_scalar_mul(negm[:], cmaxes[0][:], -SCALE)
            state[si] = (scores, negm)

        def pass_b(si):
            """exp (ACT) + xbar transpose + attn@vp (PE) + store."""
            s0 = si * P
            nch = si // 4 + 1
            ntb = si + 1
            dw = max(2 * P, (si % 4 + 1) * P)
            scores, negm = state.pop(si)

            attnT = at_pool.tile([P, NSTRIP, P], FP16, name="attnT")
            cps = ctx_ps.tile([P, 2, TCH], F32, name="ctxps", tag="ctxps")
            # all exps first (they only need negm); then the transpose of
            # chunk c+1 overlaps the AV matmuls of chunk c
            partials = []
            expcs = []
            for c in range(nch):
                cw = dw if c == nch - 1 else TCH
                expc = exp_pool.tile([P, TCH], FP16, name="expc")
                # no accum_out: its ~280ns readback would delay the attnT
                # transposes that consume expc; the DVE sums instead
                nc.scalar.activation(expc[:, :cw],
                                     scores[:, c * TCH:c * TCH + cw],
                                     mybir.ActivationFunctionType.Exp,
                                     bias=negm[:], scale=SCALE)
                part = st_pool.tile([P, 1], F32, name="part", tag="st")
                nc.vector.reduce_sum(part[:], expc[:, :cw],
                                     axis=mybir.AxisListType.X)
                partials.append(part)
                expcs.append(expc)

            def tr_chunk(c):
                nblk = min(4, ntb - 4 * c)
                ps = mm_ps.tile([P, TCH], FP16, name="tps2", tag="mmps")
                for g in range(nblk):
                    nc.tensor.transpose(ps[:, g * P:(g + 1) * P],
                                        expcs[c][:, g * P:(g + 1) * P],
                                        ident16[:])
                # ACT (not DVE) copy: keeps the exp->transpose->AV chain off
                # the DVE queue, which is busy with the next strip's pass A
                nc.scalar.activation(
                    attnT[:, 4 * c:4 * c + nblk, :],
                    ps[:, :nblk * P].rearrange("p (c s) -> p c s", c=nblk),
                    mybir.ActivationFunctionType.Copy)

            tr_chunk(0)
            for c in range(nch):
                if c + 1 < nch:
                    tr_chunk(c + 1)
                nblk = min(4, ntb - 4 * c)
                for tb in range(4 * c, 4 * c + nblk):
                    for ih in range(2):
                        nc.tensor.matmul(cps[:, ih, :],
                                         attnT[:, tb, :],
                                         val[:, tb, ih * TCH:(ih + 1) * TCH],
                                         start=(tb == 0),
                                         stop=(tb == ntb - 1))
            rowsum = st_pool.tile([P, 1], F32, name="rowsum", tag="st")
            if len(partials) == 1:
                nc.vector.tensor_copy(rowsum[:], partials[0][:])
            else:
                nc.vector.tensor_add(rowsum[:], partials[0][:], partials[1][:])
                for part in partials[2:]:
                    nc.vector.tensor_add(rowsum[:], rowsum[:], part[:])
            recip = st_pool.tile([P, 1], F32, name="recip", tag="st")
            nc.vector.reciprocal(recip[:], rowsum[:])
            # normalize + store in two halves so the first DMA overlaps the
            # second half's ACT copy (shortens the wind-down tail)
            ob = ob_pool.tile([P, D], F32, name="ob")
            for ih in range(2):
                nc.scalar.activation(ob[:, ih * TCH:(ih + 1) * TCH],
                                     cps[:, ih, :],
                                     mybir.ActivationFunctionType.Copy,
                                     scale=recip[:])
                nc.sync.dma_start(
                    out_d.ap()[s0:s0 + P, ih * TCH:(ih + 1) * TCH],
                    ob[:, ih * TCH:(ih + 1) * TCH])

        # ---- strip loop: descending order (big strips first, so the
        # wind-down tail is the cheapest strip); pipeline depth 2, deepening
        # to 3 for the small tail strips whose serial chains dominate ----
        # strip 2 (tiny) first: its short softmax chain fills the pipeline
        # right after the prologue; then big-to-small so the wind-down strip
        # is the cheapest
        order = [2] + [s for s in range(NSTRIP - 1, -1, -1) if s != 2]
        prefetch_q(order[0])
        prefetch_q(order[1])
        prefetch_q(order[2])
        pass_a(order[0])
        pass_a(order[1])
        pass_a(order[2])
        nxt = 3
        for i, si in enumerate(order):
            want = nxt + 1 if i >= 9 else nxt
            for k in range(nxt, min(want + 1, NSTRIP)):
                prefetch_q(order[k])
            pass_b(si)
            for k in range(nxt, min(want + 1, NSTRIP)):
                pass_a(order[k])
                nxt = k + 1

    _mark(nc, 'end')
    nc.finalize()
    return nc


_NC_CACHE = None


def kernel(**inputs):
    global _NC_CACHE
    if _NC_CACHE is None:
        _NC_CACHE = build_nc()
    nc = _NC_CACHE
    query = np.ascontiguousarray(inputs["query"], dtype=np.float32)
    key = np.ascontiguousarray(inputs["key"], dtype=np.float32)
    value = np.ascontiguousarray(inputs["value"], dtype=np.float32)
    Wq = np.ascontiguousarray(inputs["Wq"], dtype=np.float32)
    Wk = np.ascontiguousarray(inputs["Wk"], dtype=np.float32)
    Wv = np.ascontiguousarray(inputs["Wv"], dtype=np.float32)
    in_maps = [
        {"query": query[i], "key": key[i], "value": value[i],
         "Wq": Wq, "Wk": Wk, "Wv": Wv}
        for i in range(N)
    ]
    res = run_bass_kernel_spmd(nc, in_maps, core_ids=list(range(N)))
    return np.stack([res.results[i]["out"] for i in range(N)], axis=0)
